# revision 27
# baseline (speedup 1.0000x reference)
"""Trainium2 Bass kernel for nn_Attention2d (sparse_attention).

Reference computation (per batch b=1):
    qkv = pair_act @ W_qkv.T + b_qkv              # [S,R,3D]
    q,k,v split, reshaped to heads [S,R,H,HD]
    logits[s,h,i,j] = q[s,i,h,:] . k[s,j,h,:]
    logits masked with attention_mask[s,j] -> -1e9
    attn = softmax_j(logits) * bias[h,i,j]
    o[s,i,:] = sum_j attn . v  -> out = o @ W_out.T + b_out

Sharding: data-parallel over S (32 rows -> 4 per core, 8 cores), no
collectives.

Per-core structure (cost-model-driven: PE matmul time = moving columns
only; LdWeights is free on the engine; ACT exp of the 8x384x384 logits
is the hard floor, so everything else hides in its shadow):
  - All operand transposes precomputed ON HOST; inputs arrive as flat
    bf16/f32 DMA loads spread across the SP/ACT/Pool queues.
  - logits^T[j,i] per head via k-stationary matmuls (3x384 cols),
    emitted at scheduler priority 0 so ACT never waits on them.
  - softmax denominators: P^T chunks STATIONARY, mask column moving ->
    1-column matmuls (engine-free).
  - o[i,d]: P*bias chunks STATIONARY, masked v columns moving (32 cols
    per head) -> 4x fewer PE columns than streaming P.
  - o normalized (DVE, bf16) then PE-transposed via a bf16 identity into
    a bf16-bitcast view of a spare PSUM bank; out-proj consumes the bf16
    copy; fp32 results DMA out per row.
  - PSUM (8 banks): logits ring 2x3 banks; 1 bank for o-chunk ic0 +
    denominators + out-proj; 1 "ring" bank time-shared between qkv
    projection chunks (which trickle between attention heads) and the
    row epilogues (o-chunks ic1/ic2, then the transposed-o slots).
  - PE p-state: warm-up filler matmuls run during the initial DMA wait
    so every real matmul executes at the full 2.4 GHz rate.
"""

import ml_dtypes
import numpy as np

import concourse.bass as bass
import concourse.tile as tile
import concourse.mybir as mybir
from concourse import bacc
from concourse.bass_utils import run_bass_kernel_spmd

# Problem shapes (hardcoded per contract; kernel.py must be self-contained).
B, S, R, D = 1, 32, 384, 256
H, HD = 8, 32
NCORES = 8
SS = S // NCORES          # 4 sequence rows per core
M = SS * R                # 1536 flattened rows per core
JT = R // 128             # 3 j-chunks per row
KT = D // 128             # 2 k-chunks of the model dim
F32 = mybir.dt.float32
BF16 = mybir.dt.bfloat16
AF = mybir.ActivationFunctionType
ALU = mybir.AluOpType

# bf16 flat tensor layout (bf16 elements), all segments [128, F]:
WQA_F = 2 * 768                          # wqT
XB0_F = 2 * 384 + SS * JT                # xT-row0 | m01(bf16)
WX1_F = 3 * 2 * 384 + 2 * 256 + 128      # xT-rows1-3 | woT | ident
BIAS_HALF_F = 4 * JT * 384               # 4 heads of biasT
BOFF_WQA = 0
BOFF_XB0 = BOFF_WQA + 128 * WQA_F
BOFF_WX1 = BOFF_XB0 + 128 * XB0_F
BOFF_BIASA = BOFF_WX1 + 128 * WX1_F
BOFF_BIASB = BOFF_BIASA + 128 * BIAS_HALF_F
NBF = BOFF_BIASB + 128 * BIAS_HALF_F
# f32 flat tensor layout:
F32S_F = SS * JT + 6                     # m01(f32) | bq
FOFF_F32S = 0
FOFF_BVBO = FOFF_F32S + 128 * F32S_F     # [1, 512]: b_v | b_out
NF32 = FOFF_BVBO + 512


def build_program(zero_bias: bool = False) -> bass.Bass:
    nc = bacc.Bacc("TRN2", target_bir_lowering=False, debug=False,
                   num_devices=NCORES)
    allin_bf = nc.dram_tensor("allin_bf", [NBF], BF16, kind="ExternalInput")
    allin_f32 = nc.dram_tensor("allin_f32", [NF32], F32,
                               kind="ExternalInput")
    out_dram = nc.dram_tensor("out", [M, D], F32, kind="ExternalOutput")
    with tile.TileContext(nc) as tc:
        _emit(nc, tc, allin_bf, allin_f32, out_dram, zero_bias)
    nc.compile()
    return nc


def _emit(nc, tc, allin_bf, allin_f32, out_dram, zero_bias):
    from contextlib import ExitStack
    ctx = ExitStack()
    with ctx:
        singles = ctx.enter_context(tc.tile_pool(name="singles", bufs=1))

        wqa = singles.tile([128, WQA_F], BF16)
        xb0 = singles.tile([128, XB0_F], BF16)
        wx1 = singles.tile([128, WX1_F], BF16)
        f32s = singles.tile([128, F32S_F], F32)
        biasT = singles.tile([128, H, JT, 384], BF16)
        qkT = singles.tile([128, 4, M], BF16)
        vsb = singles.tile([128, SS * JT, D], BF16)
        fill = singles.tile([128, 512], BF16)
        scratch = singles.tile([128, 8], BF16)

        # views into the flat DMA tiles
        wqT = wqa[:].rearrange("p (kt n) -> p kt n", kt=2)
        xTr = [xb0[:, 0:768].rearrange("p (kt m) -> p kt m", kt=2)]
        for r_ in range(3):
            xTr.append(wx1[:, r_ * 768:(r_ + 1) * 768]
                       .rearrange("p (kt m) -> p kt m", kt=2))
        m01b = xb0[:, 768:768 + SS * JT].rearrange(
            "p (s jt) -> p s jt", s=SS)
        woT = wx1[:, 2304:2816].rearrange("p (kt n) -> p kt n", kt=2)
        identb = wx1[:, 2816:2944]
        m01f = f32s[:, 0:SS * JT].rearrange("p (s jt) -> p s jt", s=SS)
        bq = f32s[:, SS * JT:SS * JT + 6]

        # warm-up fodder for the PE p-state ramp
        nc.vector.memset(fill[:], 1.0)

        # ---- input DMAs, spread across queues so nothing head-blocks ----
        # ACT: the x/mask segment (critical path), then the act-table warm-up
        nc.scalar.dma_start(
            out=xb0[:], in_=allin_bf[BOFF_XB0:BOFF_WX1]
            .rearrange("(p f) -> p f", f=XB0_F))
        nc.scalar.activation(scratch[:], fill[:, 0:8], AF.Exp)
        # SP: weights first, then the bias tensor in quarters with floors
        # so they never hog the DMA bus ahead of more urgent loads
        nc.sync.dma_start(
            out=wqa[:], in_=allin_bf[BOFF_WQA:BOFF_XB0]
            .rearrange("(p f) -> p f", f=WQA_F))
        bias_flat = allin_bf[BOFF_BIASA:NBF].rearrange(
            "(p f) -> p f", f=2 * BIAS_HALF_F)
        for i, floor_ns in enumerate((2500, 4200, 6200)):
            with tc.tile_wait_until(floor_ns * 1e-6):
                nc.sync.dma_start(
                    out=biasT[:, 2 * i:2 * i + 2],
                    in_=bias_flat[:, i * 2304:(i + 1) * 2304])
        # Pool (SWDGE): small f32 scalars, the rest of x + out-proj weights,
        # and the last bias quarter
        with tc.tile_wait_until(2000 * 1e-6):
            nc.gpsimd.dma_start(
                out=f32s[:], in_=allin_f32[FOFF_F32S:FOFF_BVBO]
                .rearrange("(p f) -> p f", f=F32S_F))
        with tc.tile_wait_until(4000 * 1e-6):
            nc.gpsimd.dma_start(
                out=wx1[:], in_=allin_bf[BOFF_WX1:BOFF_BIASA]
                .rearrange("(p f) -> p f", f=WX1_F))
        with tc.tile_wait_until(7800 * 1e-6):
            nc.gpsimd.dma_start(
                out=biasT[:, 6:8], in_=bias_flat[:, 3 * 2304:4 * 2304])
        if not zero_bias:
            bvbo = singles.tile([1, 512], F32)
            bvbo_bf = singles.tile([1, 512], BF16)
            ones1 = singles.tile([1, 128], BF16)
            bv_bc = singles.tile([128, D], F32)
            bo_bc = singles.tile([128, D], F32)
            nc.sync.dma_start(
                out=bvbo[:], in_=allin_f32[FOFF_BVBO:FOFF_BVBO + 512]
                .rearrange("(p f) -> p f", p=1))
            nc.vector.tensor_copy(bvbo_bf[:], bvbo[:])
            nc.vector.memset(ones1[:], 1.0)

        # ---- pre-phase: PE warm-up + the first two q/k chunks ----
        def qk_copy(engine, nt, dst, src):
            if zero_bias:
                engine.tensor_copy(dst, src)
            else:
                engine.tensor_scalar_add(dst, src, bq[:, nt:nt + 1])

        with tc.tile_pool(name="pre", bufs=2, space="PSUM") as pre:
            for _ in range(6):
                pf_ = pre.tile([128, 512], F32, tag="pre", name="fillp")
                nc.tensor.matmul(pf_[:], fill[:, 0:128], fill[:],
                                 start=True, stop=True)
            if not zero_bias:
                pb = pre.tile([128, 512], F32, tag="pre", name="bvp")
                nc.tensor.matmul(pb[:, 0:256], ones1[:], bvbo_bf[0:1, 0:256],
                                 start=True, stop=True)
                nc.tensor.matmul(pb[:, 256:512], ones1[:],
                                 bvbo_bf[0:1, 256:512],
                                 start=True, stop=True)
                nc.vector.tensor_copy(bv_bc[:], pb[:, 0:256])
                nc.vector.tensor_copy(bo_bc[:], pb[:, 256:512])
            for nt in (0, 2):
                pq = pre.tile([128, 512], F32, tag="pre", name="preqk")
                for kt in range(KT):
                    nc.tensor.matmul(pq[:, 0:384],
                                     wqT[:, kt, nt * 128:(nt + 1) * 128],
                                     xTr[0][:, kt, :],
                                     start=(kt == 0), stop=(kt == KT - 1))
                if nt == 2 and zero_bias:
                    nc.scalar.copy(qkT[:, nt, 0:384], pq[:, 0:384])
                else:
                    qk_copy(nc.vector, nt, qkT[:, nt, 0:384], pq[:, 0:384])

        # ---- main pools ----
        lgp = ctx.enter_context(
            tc.tile_pool(name="lg", bufs=2, space="PSUM"))
        pop = ctx.enter_context(
            tc.tile_pool(name="po", bufs=1, space="PSUM"))
        ringp = ctx.enter_context(
            tc.tile_pool(name="ring", bufs=1, space="PSUM"))
        ptp = ctx.enter_context(tc.tile_pool(name="pt", bufs=3))
        pbtp = ctx.enter_context(tc.tile_pool(name="pbt", bufs=10))
        onp = ctx.enter_context(tc.tile_pool(name="onrm", bufs=3))
        otp = ctx.enter_context(tc.tile_pool(name="otsb", bufs=2))
        recfp = ctx.enter_context(tc.tile_pool(name="recf", bufs=3))
        recep = ctx.enter_context(tc.tile_pool(name="rece", bufs=3))
        fop = ctx.enter_context(tc.tile_pool(name="fo", bufs=2))

        # ---- dribble: remaining qkv chunks, between attention heads ----
        def emit_qk(nt, row, eng):
            rt = ringp.tile([128, 512], F32, tag="ring", name="rqk")
            for kt in range(KT):
                nc.tensor.matmul(rt[:, 0:384],
                                 wqT[:, kt, nt * 128:(nt + 1) * 128],
                                 xTr[row][:, kt, :],
                                 start=(kt == 0), stop=(kt == KT - 1))
            qk_copy(eng, nt, qkT[:, nt, row * R:(row + 1) * R], rt[:, 0:384])

        def emit_v(mt, eng):
            s_, jt_ = mt // JT, mt % JT
            rt = ringp.tile([128, 512], F32, tag="ring", name="rv")
            for kt in range(KT):
                nc.tensor.matmul(rt[:, 0:256],
                                 xTr[s_][:, kt, jt_ * 128:(jt_ + 1) * 128],
                                 wqT[:, kt, 512:768],
                                 start=(kt == 0), stop=(kt == KT - 1))
            if zero_bias:
                nc.vector.tensor_scalar_mul(
                    vsb[:, mt, :], rt[:, 0:256], m01f[:, s_, jt_:jt_ + 1])
            else:
                nc.vector.scalar_tensor_tensor(
                    vsb[:, mt, :], rt[:, 0:256], m01f[:, s_, jt_:jt_ + 1],
                    bv_bc[:], ALU.mult, ALU.add)

        def qk_(nt, mc, eng=None):
            return lambda: emit_qk(nt, mc, eng or nc.vector)

        def v_(mt, eng=None):
            return lambda: emit_v(mt, eng or nc.vector)

        # drip schedule per (row, head): early copies go to DVE because the
        # Pool queue is still busy issuing the bias-tensor SWDGE loads
        drip_sched = {
            (0, 0): [qk_(1, 0, nc.vector)], (0, 1): [qk_(3, 0, nc.vector)],
            (0, 2): [qk_(0, 1, nc.vector)], (0, 3): [qk_(2, 1, nc.vector)],
            (0, 4): [v_(0, nc.vector)], (0, 5): [v_(1, nc.vector)],
            (0, 6): [v_(2, nc.vector)], (0, 7): [qk_(1, 1)],
            (1, 0): [qk_(3, 1)], (1, 2): [qk_(0, 2)], (1, 3): [qk_(2, 2)],
            (1, 5): [v_(3)], (1, 6): [v_(4)], (1, 7): [v_(5)],
            (2, 0): [qk_(1, 2), qk_(3, 2)], (2, 2): [v_(6)],
            (2, 3): [qk_(0, 3)], (2, 4): [qk_(2, 3)],
            (2, 5): [v_(7)], (2, 6): [v_(8)], (2, 7): [qk_(1, 3)],
            (3, 0): [qk_(3, 3)], (3, 2): [v_(9)], (3, 3): [v_(10)],
            (3, 4): [v_(11)],
        }
        EXP_NS, ROW_NS, T0_NS = 1145, 9600, 6500

        def slot_floor(s, h):
            return (T0_NS + s * ROW_NS + h * EXP_NS - 300) * 1e-6

        # ---- row epilogue ----
        def burst(tgt, pbts, s, ic, h0=0, h1=H):
            """o-chunk ic for row s: P*bias stationary, masked v moving."""
            for h in range(h0, h1):
                for jt in range(JT):
                    nc.tensor.matmul(
                        tgt[:, h * HD:(h + 1) * HD],
                        pbts[h][:, jt, ic * 128:(ic + 1) * 128],
                        vsb[:, s * JT + jt, h * HD:(h + 1) * HD],
                        start=(jt == 0), stop=(jt == JT - 1))

        def make_tail(s, po_t, pbts, recs, on0):
            box = {}

            def t_bursts():
                rb = ringp.tile([128, 512], F32, tag="ring", name="rb")
                box["rb"] = rb
                burst(rb[:, 0:256], pbts, s, 1)
                burst(rb[:, 256:512], pbts, s, 2)

            def t_norms():
                rb = box["rb"]
                ons = [on0]
                for ic, src in ((1, rb[:, 0:256]), (2, rb[:, 256:512])):
                    on = onp.tile([128, D], BF16, tag="on", name="on")
                    nc.vector.tensor_mul(on[:], src, recs[ic][:])
                    ons.append(on)
                box["ons"] = ons

            def t_transp(po_cur):
                ons = box["ons"]
                slots_a = po_cur[:, 0:256].bitcast(BF16)
                slots_b = po_cur[:, 280:408].bitcast(BF16)

                def slot(sl):
                    if sl < 4:
                        return slots_a[:, sl * 128:(sl + 1) * 128]
                    return slots_b[:, (sl - 4) * 128:(sl - 3) * 128]

                for ic in range(3):
                    for dk in range(KT):
                        nc.tensor.transpose(
                            slot(ic * 2 + dk),
                            ons[ic][:, dk * 128:(dk + 1) * 128], identb[:])
                ot = otp.tile([128, 3, KT, 128], BF16, tag="ot", name="ot")
                nc.vector.tensor_copy(
                    ot[:, 0:2].rearrange("p a b c -> p (a b c)"), slots_a[:])
                nc.vector.tensor_copy(
                    ot[:, 2].rearrange("p a b -> p (a b)"), slots_b[:])
                box["ot"] = ot
                box["fo"] = fop.tile([128, 3, D], F32, tag="fo", name="fo")

            def t_pf(ic, po_cur):
                ot, fo = box["ot"], box["fo"]
                pf = po_cur[:, 0:256]
                for dk in range(KT):
                    nc.tensor.matmul(pf, ot[:, ic, dk, :], woT[:, dk, :],
                                     start=(dk == 0), stop=(dk == KT - 1))
                if zero_bias:
                    nc.vector.tensor_copy(fo[:, ic, :], pf)
                else:
                    nc.vector.scalar_tensor_tensor(
                        fo[:, ic, :], pf, 1.0, bo_bc[:], ALU.mult, ALU.add)

            def t_dma():
                nc.sync.dma_start(
                    out=out_dram[s * R:(s + 1) * R, :]
                    .rearrange("(ic p) n -> p ic n", p=128),
                    in_=box["fo"][:])

            return {0: lambda po_cur: t_bursts(),
                    1: lambda po_cur: t_norms(),
                    2: lambda po_cur: t_transp(po_cur),
                    3: lambda po_cur: t_pf(0, po_cur),
                    4: lambda po_cur: t_pf(1, po_cur),
                    5: lambda po_cur: t_pf(2, po_cur),
                    6: lambda po_cur: t_dma()}, box

        # ---- attention rows ----
        def emit_L(s, h):
            b, g = 32 * (h % 4), h // 4
            lgt = lgp.tile([128, JT, 512], F32, tag="lg", name="lgt")
            for jt in range(JT):
                nc.tensor.matmul(
                    lgt[:, jt, 0:R],
                    qkT[b:b + 32, 2 + g,
                        s * R + jt * 128:s * R + (jt + 1) * 128],
                    qkT[b:b + 32, g, s * R:(s + 1) * R],
                    start=True, stop=True, tile_position=(b, 0))
            return lgt

        tail_sched, tail_box = {}, None
        lgt_next = emit_L(0, 0)
        for s in range(SS):
            po_t = pop.tile([128, 512], F32, tag="po", name="po_t")
            pbts = []
            for h in range(H):
                lgt = lgt_next
                pt = ptp.tile([128, JT, R], BF16, tag="pt", name="pt")
                nc.scalar.activation(pt[:], lgt[:, :, 0:R], AF.Exp)
                if (s, h) != (SS - 1, H - 1):
                    ns, nh = (s, h + 1) if h < H - 1 else (s + 1, 0)
                    lgt_next = emit_L(ns, nh)
                pbt = pbtp.tile([128, JT, R], BF16, tag="pbt", name="pbt")
                pbt_eng = nc.gpsimd if h in (1, 3, 5) else nc.vector
                pbt_eng.tensor_mul(pbt[:], pt[:], biasT[:, h])
                for ic in range(3):
                    col = 256 + ic * 8 + h
                    for jt in range(JT):
                        nc.tensor.matmul(
                            po_t[:, col:col + 1],
                            pt[:, jt, ic * 128:(ic + 1) * 128],
                            m01b[:, s, jt:jt + 1],
                            start=(jt == 0), stop=(jt == JT - 1))
                pbts.append(pbt)
                with tc.tile_wait_until(slot_floor(s, h)):
                    for fn in drip_sched.get((s, h), ()):
                        fn()
                    if h in tail_sched:
                        tail_sched.pop(h)(po_t)
                    if s == SS - 1 and h == 5:
                        rb3 = ringp.tile([128, 512], F32, tag="ring",
                                         name="rb3")
                        burst(po_t[:, 0:256], pbts, s, 0, 0, 6)
                        burst(rb3[:, 0:256], pbts, s, 1, 0, 6)
                        burst(rb3[:, 256:512], pbts, s, 2, 0, 6)
                    if s == SS - 1 and h == 6:
                        burst(po_t[:, 0:256], pbts, s, 0, 6, 7)
                        burst(rb3[:, 0:256], pbts, s, 1, 6, 7)
                        burst(rb3[:, 256:512], pbts, s, 2, 6, 7)
            # row end: reciprocals, o-chunk ic0, its normalize
            rf = recfp.tile([128, 24], F32, tag="rf", name="rf")
            nc.vector.reciprocal(rf[:], po_t[:, 256:280])
            recs = []
            for ic in range(3):
                re = recep.tile([128, D], F32, tag="re", name="re")
                nc.gpsimd.tensor_copy(
                    re[:].rearrange("p (h t) -> p h t", t=HD),
                    rf[:, ic * 8:(ic + 1) * 8]
                    .rearrange("p (h o) -> p h o", o=1)
                    .broadcast_to((128, 8, HD)))
                recs.append(re)
            if s == SS - 1:
                burst(po_t[:, 0:256], pbts, s, 0, 7, 8)
            else:
                burst(po_t[:, 0:256], pbts, s, 0)
            on0 = onp.tile([128, D], BF16, tag="on", name="on0")
            nc.vector.tensor_mul(on0[:], po_t[:, 0:256], recs[0][:])
            assert not tail_sched
            tail_sched, tail_box = make_tail(s, po_t, pbts, recs, on0)
            if s == SS - 1:
                # last row: finish bursts (head 7), then per-ic chains with
                # per-engine queues ordered by expected readiness
                tail_sched = {}
                rb = rb3
                burst(rb[:, 0:256], pbts, s, 1, 7, 8)
                burst(rb[:, 256:512], pbts, s, 2, 7, 8)
                srcs = (po_t[:, 0:256], rb[:, 0:256], rb[:, 256:512])
                ons = [on0]
                for ic_ in (1, 2):
                    on_ = onp.tile([128, D], BF16, tag="on", name="onl")
                    nc.vector.tensor_mul(on_[:], srcs[ic_], recs[ic_][:])
                    ons.append(on_)
                Tl = lgp.tile([128, JT, 512], F32, tag="lg", name="Tl")
                slots = Tl[:, 0, :].bitcast(BF16)
                ot = otp.tile([128, 3, KT, 128], BF16, tag="ot", name="otl")
                fo = fop.tile([128, 3, D], F32, tag="fo", name="fol")
                pf_tgts = (Tl[:, 1, 0:256], Tl[:, 2, 0:256], po_t[:, 0:256])

                def transp(ic):
                    for dk in range(KT):
                        sl = ic * 2 + dk
                        nc.tensor.transpose(
                            slots[:, sl * 128:(sl + 1) * 128],
                            ons[ic][:, dk * 128:(dk + 1) * 128], identb[:])

                def otcopy(ic, eng):
                    otv = ot[:, ic].rearrange("p a b -> p (a b)")
                    if eng is nc.scalar:
                        eng.copy(otv, slots[:, ic * 256:(ic + 1) * 256])
                    else:
                        eng.tensor_copy(otv, slots[:, ic * 256:(ic + 1) * 256])

                def pf_mm(ic):
                    for dk in range(KT):
                        nc.tensor.matmul(pf_tgts[ic], ot[:, ic, dk, :],
                                         woT[:, dk, :],
                                         start=(dk == 0), stop=(dk == KT - 1))

                def fo_cp(ic, eng):
                    if not zero_bias:
                        eng.scalar_tensor_tensor(
                            fo[:, ic, :], pf_tgts[ic], 1.0, bo_bc[:],
                            ALU.mult, ALU.add)
                    elif eng is nc.scalar:
                        eng.copy(fo[:, ic, :], pf_tgts[ic])
                    else:
                        eng.tensor_copy(fo[:, ic, :], pf_tgts[ic])

                def dma(ic, eng):
                    eng.dma_start(
                        out=out_dram[(s * JT + ic) * 128:
                                     (s * JT + ic + 1) * 128, :],
                        in_=fo[:, ic, :])

                # PE: all transposes first, then the out-proj matmuls
                transp(0)
                transp(1)
                transp(2)
                otcopy(0, nc.scalar)    # ACT (free after the last exp)
                otcopy(1, nc.scalar)
                otcopy(2, nc.scalar)
                pf_mm(0)
                pf_mm(1)
                pf_mm(2)
                fo_cp(0, nc.scalar)
                fo_cp(1, nc.vector)
                fo_cp(2, nc.scalar)
                dma(0, nc.gpsimd)
                dma(1, nc.sync)
                dma(2, nc.scalar)
def make_in_maps(pair_act, attention_mask, bias, W_qkv, b_qkv, W_out, b_out):
    """Shard the full inputs across the 8 cores (data-parallel over S).

    All matmul operand transposes happen here on the host; the device sees
    flat [128, F] segments it can DMA without any on-device transposes.
    """
    bf = ml_dtypes.bfloat16
    x_all = np.asarray(pair_act, np.float32)[0]          # [S, R, D]
    mask01 = 1.0 - np.asarray(attention_mask, np.float32)[0]  # [S, R]
    b3 = np.asarray(bias, np.float32)[0, 0]              # [H, R, R]
    Wq = np.asarray(W_qkv, np.float32)                   # [3D, D]
    Wo = np.asarray(W_out, np.float32)                   # [D, D]

    wqT = Wq.T.reshape(KT, 128, 3 * D).transpose(1, 0, 2)    # [128,2,768]
    woT = Wo.T.reshape(KT, 128, D).transpose(1, 0, 2)        # [128,2,256]
    biasT = (b3.transpose(0, 2, 1)                           # [H, j, i]
             .reshape(H, JT, 128, R).transpose(2, 0, 1, 3))  # [128,H,3,384]
    bias_all = biasT.astype(bf).reshape(128, -1).ravel()  # [128, H*3*384]
    wq_bf = wqT.astype(bf).reshape(128, -1)
    wo_bf = woT.astype(bf).reshape(128, -1)
    identb = np.eye(128, dtype=np.float32).astype(bf)
    bqv = np.asarray(b_qkv, np.float32)
    bq6 = bqv[0:768].reshape(6, 128).T                       # [128, 6]

    in_maps = []
    for c in range(NCORES):
        xs = x_all[c * SS:(c + 1) * SS].reshape(M, D)        # [1536, 256]
        xT = xs.T.reshape(KT, 128, M).transpose(1, 0, 2)     # [128, 2, 1536]
        xrow = [xT[:, :, r_ * R:(r_ + 1) * R].astype(bf).reshape(128, -1)
                for r_ in range(SS)]
        m01 = mask01[c * SS:(c + 1) * SS]                    # [4, 384]
        m01p = m01.reshape(SS, JT, 128).transpose(2, 0, 1)   # [128, 4, 3]
        xb0 = np.concatenate([xrow[0],
                              m01p.astype(bf).reshape(128, -1)], axis=1)
        wx1 = np.concatenate(
            [xrow[1], xrow[2], xrow[3], wo_bf, identb], axis=1)
        allin_bf = np.concatenate(
            [wq_bf.ravel(), xb0.ravel(), wx1.ravel(), bias_all])
        f32seg = np.concatenate([m01p.reshape(128, -1), bq6], axis=1)
        allin_f32 = np.concatenate([
            f32seg.ravel().astype(np.float32),
            bqv[512:768], np.asarray(b_out, np.float32)])
        assert allin_bf.size == NBF and allin_f32.size == NF32, \
            (allin_bf.size, NBF, allin_f32.size, NF32)
        in_maps.append({
            "allin_bf": np.ascontiguousarray(allin_bf),
            "allin_f32": np.ascontiguousarray(allin_f32.astype(np.float32)),
        })
    return in_maps


_PROGRAM_CACHE = {}


def kernel(pair_act, attention_mask, bias, W_qkv, b_qkv, W_out, b_out,
           _want_results=False, **extra):
    in_maps = make_in_maps(pair_act, attention_mask, bias, W_qkv, b_qkv,
                           W_out, b_out)
    zero_bias = bool(np.all(np.asarray(b_qkv) == 0)
                     and np.all(np.asarray(b_out) == 0))
    key = ("nc", zero_bias)
    if key not in _PROGRAM_CACHE:
        _PROGRAM_CACHE[key] = build_program(zero_bias)
    nc = _PROGRAM_CACHE[key]
    res = run_bass_kernel_spmd(nc, in_maps, core_ids=list(range(NCORES)))
    out = np.concatenate(
        [r["out"].reshape(SS, R, D) for r in res.results], axis=0)
    out = out.reshape(B, S, R, D).astype(np.float32)
    if _want_results:
        return out, res
    return out


# revision 28
# speedup vs baseline: 1.0019x; 1.0019x over previous
"""Trainium2 Bass kernel for nn_Attention2d (sparse_attention).

Reference computation (per batch b=1):
    qkv = pair_act @ W_qkv.T + b_qkv              # [S,R,3D]
    q,k,v split, reshaped to heads [S,R,H,HD]
    logits[s,h,i,j] = q[s,i,h,:] . k[s,j,h,:]
    logits masked with attention_mask[s,j] -> -1e9
    attn = softmax_j(logits) * bias[h,i,j]
    o[s,i,:] = sum_j attn . v  -> out = o @ W_out.T + b_out

Sharding: data-parallel over S (32 rows -> 4 per core, 8 cores), no
collectives.

Per-core structure (cost-model-driven: PE matmul time = moving columns
only; LdWeights is free on the engine; ACT exp of the 8x384x384 logits
is the hard floor, so everything else hides in its shadow):
  - All operand transposes precomputed ON HOST; inputs arrive as flat
    bf16/f32 DMA loads spread across the SP/ACT/Pool queues.
  - logits^T[j,i] per head via k-stationary matmuls (3x384 cols),
    emitted at scheduler priority 0 so ACT never waits on them.
  - softmax denominators: P^T chunks STATIONARY, mask column moving ->
    1-column matmuls (engine-free).
  - o[i,d]: P*bias chunks STATIONARY, masked v columns moving (32 cols
    per head) -> 4x fewer PE columns than streaming P.
  - o normalized (DVE, bf16) then PE-transposed via a bf16 identity into
    a bf16-bitcast view of a spare PSUM bank; out-proj consumes the bf16
    copy; fp32 results DMA out per row.
  - PSUM (8 banks): logits ring 2x3 banks; 1 bank for o-chunk ic0 +
    denominators + out-proj; 1 "ring" bank time-shared between qkv
    projection chunks (which trickle between attention heads) and the
    row epilogues (o-chunks ic1/ic2, then the transposed-o slots).
  - PE p-state: warm-up filler matmuls run during the initial DMA wait
    so every real matmul executes at the full 2.4 GHz rate.
"""

import ml_dtypes
import numpy as np

import concourse.bass as bass
import concourse.tile as tile
import concourse.mybir as mybir
from concourse import bacc
from concourse.bass_utils import run_bass_kernel_spmd

# Problem shapes (hardcoded per contract; kernel.py must be self-contained).
B, S, R, D = 1, 32, 384, 256
H, HD = 8, 32
NCORES = 8
SS = S // NCORES          # 4 sequence rows per core
M = SS * R                # 1536 flattened rows per core
JT = R // 128             # 3 j-chunks per row
KT = D // 128             # 2 k-chunks of the model dim
F32 = mybir.dt.float32
BF16 = mybir.dt.bfloat16
AF = mybir.ActivationFunctionType
ALU = mybir.AluOpType

# bf16 flat tensor layout (bf16 elements), all segments [128, F]:
WQA_F = 2 * 768                          # wqT
XB0_F = 2 * 384 + SS * JT                # xT-row0 | m01(bf16)
WX1_F = 3 * 2 * 384 + 2 * 256 + 128      # xT-rows1-3 | woT | ident
BIAS_HALF_F = 4 * JT * 384               # 4 heads of biasT
BOFF_WQA = 0
BOFF_XB0 = BOFF_WQA + 128 * WQA_F
BOFF_WX1 = BOFF_XB0 + 128 * XB0_F
BOFF_BIASA = BOFF_WX1 + 128 * WX1_F
BOFF_BIASB = BOFF_BIASA + 128 * BIAS_HALF_F
NBF = BOFF_BIASB + 128 * BIAS_HALF_F
# f32 flat tensor layout:
F32S_F = SS * JT + 6                     # m01(f32) | bq
FOFF_F32S = 0
FOFF_BVBO = FOFF_F32S + 128 * F32S_F     # [1, 512]: b_v | b_out
NF32 = FOFF_BVBO + 512


def build_program(zero_bias: bool = False) -> bass.Bass:
    nc = bacc.Bacc("TRN2", target_bir_lowering=False, debug=False,
                   num_devices=NCORES)
    allin_bf = nc.dram_tensor("allin_bf", [NBF], BF16, kind="ExternalInput")
    allin_f32 = nc.dram_tensor("allin_f32", [NF32], F32,
                               kind="ExternalInput")
    out_dram = nc.dram_tensor("out", [M, D], F32, kind="ExternalOutput")
    with tile.TileContext(nc) as tc:
        _emit(nc, tc, allin_bf, allin_f32, out_dram, zero_bias)
    nc.compile()
    return nc


def _emit(nc, tc, allin_bf, allin_f32, out_dram, zero_bias):
    from contextlib import ExitStack
    ctx = ExitStack()
    with ctx:
        singles = ctx.enter_context(tc.tile_pool(name="singles", bufs=1))

        wqa = singles.tile([128, WQA_F], BF16)
        xb0 = singles.tile([128, XB0_F], BF16)
        wx1 = singles.tile([128, WX1_F], BF16)
        f32s = singles.tile([128, F32S_F], F32)
        biasT = singles.tile([128, H, JT, 384], BF16)
        qkT = singles.tile([128, 4, M], BF16)
        vsb = singles.tile([128, SS * JT, D], BF16)
        fill = singles.tile([128, 512], BF16)
        scratch = singles.tile([128, 8], BF16)

        # views into the flat DMA tiles
        wqT = wqa[:].rearrange("p (kt n) -> p kt n", kt=2)
        xTr = [xb0[:, 0:768].rearrange("p (kt m) -> p kt m", kt=2)]
        for r_ in range(3):
            xTr.append(wx1[:, r_ * 768:(r_ + 1) * 768]
                       .rearrange("p (kt m) -> p kt m", kt=2))
        m01b = xb0[:, 768:768 + SS * JT].rearrange(
            "p (s jt) -> p s jt", s=SS)
        woT = wx1[:, 2304:2816].rearrange("p (kt n) -> p kt n", kt=2)
        identb = wx1[:, 2816:2944]
        m01f = f32s[:, 0:SS * JT].rearrange("p (s jt) -> p s jt", s=SS)
        bq = f32s[:, SS * JT:SS * JT + 6]

        # warm-up fodder for the PE p-state ramp
        nc.vector.memset(fill[:], 1.0)

        # ---- input DMAs, spread across queues so nothing head-blocks ----
        # ACT: the x/mask segment (critical path), then the act-table warm-up
        nc.scalar.dma_start(
            out=xb0[:], in_=allin_bf[BOFF_XB0:BOFF_WX1]
            .rearrange("(p f) -> p f", f=XB0_F))
        nc.scalar.activation(scratch[:], fill[:, 0:8], AF.Exp)
        # SP: weights first, then the bias tensor in quarters with floors
        # so they never hog the DMA bus ahead of more urgent loads
        nc.sync.dma_start(
            out=wqa[:], in_=allin_bf[BOFF_WQA:BOFF_XB0]
            .rearrange("(p f) -> p f", f=WQA_F))
        bias_flat = allin_bf[BOFF_BIASA:NBF].rearrange(
            "(p f) -> p f", f=2 * BIAS_HALF_F)
        for i, floor_ns in enumerate((2500, 4200, 6200)):
            with tc.tile_wait_until(floor_ns * 1e-6):
                nc.sync.dma_start(
                    out=biasT[:, 2 * i:2 * i + 2],
                    in_=bias_flat[:, i * 2304:(i + 1) * 2304])
        # Pool (SWDGE): small f32 scalars, the rest of x + out-proj weights,
        # and the last bias quarter
        with tc.tile_wait_until(2000 * 1e-6):
            nc.gpsimd.dma_start(
                out=f32s[:], in_=allin_f32[FOFF_F32S:FOFF_BVBO]
                .rearrange("(p f) -> p f", f=F32S_F))
        with tc.tile_wait_until(4000 * 1e-6):
            nc.gpsimd.dma_start(
                out=wx1[:], in_=allin_bf[BOFF_WX1:BOFF_BIASA]
                .rearrange("(p f) -> p f", f=WX1_F))
        with tc.tile_wait_until(7800 * 1e-6):
            nc.gpsimd.dma_start(
                out=biasT[:, 6:8], in_=bias_flat[:, 3 * 2304:4 * 2304])
        if not zero_bias:
            bvbo = singles.tile([1, 512], F32)
            bvbo_bf = singles.tile([1, 512], BF16)
            ones1 = singles.tile([1, 128], BF16)
            bv_bc = singles.tile([128, D], F32)
            bo_bc = singles.tile([128, D], F32)
            nc.sync.dma_start(
                out=bvbo[:], in_=allin_f32[FOFF_BVBO:FOFF_BVBO + 512]
                .rearrange("(p f) -> p f", p=1))
            nc.vector.tensor_copy(bvbo_bf[:], bvbo[:])
            nc.vector.memset(ones1[:], 1.0)

        # ---- pre-phase: PE warm-up + the first two q/k chunks ----
        def qk_copy(engine, nt, dst, src):
            if zero_bias:
                engine.tensor_copy(dst, src)
            else:
                engine.tensor_scalar_add(dst, src, bq[:, nt:nt + 1])

        with tc.tile_pool(name="pre", bufs=2, space="PSUM") as pre:
            for _ in range(6):
                pf_ = pre.tile([128, 512], F32, tag="pre", name="fillp")
                nc.tensor.matmul(pf_[:], fill[:, 0:128], fill[:],
                                 start=True, stop=True)
            if not zero_bias:
                pb = pre.tile([128, 512], F32, tag="pre", name="bvp")
                nc.tensor.matmul(pb[:, 0:256], ones1[:], bvbo_bf[0:1, 0:256],
                                 start=True, stop=True)
                nc.tensor.matmul(pb[:, 256:512], ones1[:],
                                 bvbo_bf[0:1, 256:512],
                                 start=True, stop=True)
                nc.vector.tensor_copy(bv_bc[:], pb[:, 0:256])
                nc.vector.tensor_copy(bo_bc[:], pb[:, 256:512])
            for nt in (0, 2):
                pq = pre.tile([128, 512], F32, tag="pre", name="preqk")
                for kt in range(KT):
                    nc.tensor.matmul(pq[:, 0:384],
                                     wqT[:, kt, nt * 128:(nt + 1) * 128],
                                     xTr[0][:, kt, :],
                                     start=(kt == 0), stop=(kt == KT - 1))
                if nt == 2 and zero_bias:
                    nc.scalar.copy(qkT[:, nt, 0:384], pq[:, 0:384])
                else:
                    qk_copy(nc.vector, nt, qkT[:, nt, 0:384], pq[:, 0:384])

        # ---- main pools ----
        lgp = ctx.enter_context(
            tc.tile_pool(name="lg", bufs=2, space="PSUM"))
        pop = ctx.enter_context(
            tc.tile_pool(name="po", bufs=1, space="PSUM"))
        ringp = ctx.enter_context(
            tc.tile_pool(name="ring", bufs=1, space="PSUM"))
        ptp = ctx.enter_context(tc.tile_pool(name="pt", bufs=3))
        pbtp = ctx.enter_context(tc.tile_pool(name="pbt", bufs=10))
        onp = ctx.enter_context(tc.tile_pool(name="onrm", bufs=3))
        otp = ctx.enter_context(tc.tile_pool(name="otsb", bufs=2))
        recfp = ctx.enter_context(tc.tile_pool(name="recf", bufs=3))
        recep = ctx.enter_context(tc.tile_pool(name="rece", bufs=3))
        fop = ctx.enter_context(tc.tile_pool(name="fo", bufs=2))

        # ---- dribble: remaining qkv chunks, between attention heads ----
        def emit_qk(nt, row, eng):
            rt = ringp.tile([128, 512], F32, tag="ring", name="rqk")
            for kt in range(KT):
                nc.tensor.matmul(rt[:, 0:384],
                                 wqT[:, kt, nt * 128:(nt + 1) * 128],
                                 xTr[row][:, kt, :],
                                 start=(kt == 0), stop=(kt == KT - 1))
            qk_copy(eng, nt, qkT[:, nt, row * R:(row + 1) * R], rt[:, 0:384])

        def emit_v(mt, eng):
            s_, jt_ = mt // JT, mt % JT
            rt = ringp.tile([128, 512], F32, tag="ring", name="rv")
            for kt in range(KT):
                nc.tensor.matmul(rt[:, 0:256],
                                 xTr[s_][:, kt, jt_ * 128:(jt_ + 1) * 128],
                                 wqT[:, kt, 512:768],
                                 start=(kt == 0), stop=(kt == KT - 1))
            if zero_bias:
                nc.vector.tensor_scalar_mul(
                    vsb[:, mt, :], rt[:, 0:256], m01f[:, s_, jt_:jt_ + 1])
            else:
                nc.vector.scalar_tensor_tensor(
                    vsb[:, mt, :], rt[:, 0:256], m01f[:, s_, jt_:jt_ + 1],
                    bv_bc[:], ALU.mult, ALU.add)

        def qk_(nt, mc, eng=None):
            return lambda: emit_qk(nt, mc, eng or nc.vector)

        def v_(mt, eng=None):
            return lambda: emit_v(mt, eng or nc.vector)

        # drip schedule per (row, head): early copies go to DVE because the
        # Pool queue is still busy issuing the bias-tensor SWDGE loads
        drip_sched = {
            (0, 0): [qk_(1, 0, nc.vector)], (0, 1): [qk_(3, 0, nc.vector)],
            (0, 2): [qk_(0, 1, nc.vector)], (0, 3): [qk_(2, 1, nc.vector)],
            (0, 4): [v_(0, nc.vector)], (0, 5): [v_(1, nc.vector)],
            (0, 6): [v_(2, nc.vector)], (0, 7): [qk_(1, 1)],
            (1, 0): [qk_(3, 1)], (1, 2): [qk_(0, 2)], (1, 3): [qk_(2, 2)],
            (1, 5): [v_(3)], (1, 6): [v_(4)], (1, 7): [v_(5)],
            (2, 0): [qk_(1, 2), qk_(3, 2)], (2, 2): [v_(6)],
            (2, 3): [qk_(0, 3)], (2, 4): [qk_(2, 3)],
            (2, 5): [v_(7)], (2, 6): [v_(8)], (2, 7): [qk_(1, 3)],
            (3, 0): [qk_(3, 3)], (3, 2): [v_(9)], (3, 3): [v_(10)],
            (3, 4): [v_(11)],
        }
        EXP_NS, ROW_NS, T0_NS = 1145, 9600, 6500

        def slot_floor(s, h):
            return (T0_NS + s * ROW_NS + h * EXP_NS - 300) * 1e-6

        # ---- row epilogue ----
        def burst(tgt, pbts, s, ic, h0=0, h1=H):
            """o-chunk ic for row s: P*bias stationary, masked v moving."""
            for h in range(h0, h1):
                for jt in range(JT):
                    nc.tensor.matmul(
                        tgt[:, h * HD:(h + 1) * HD],
                        pbts[h][:, jt, ic * 128:(ic + 1) * 128],
                        vsb[:, s * JT + jt, h * HD:(h + 1) * HD],
                        start=(jt == 0), stop=(jt == JT - 1))

        def make_tail(s, po_t, pbts, recs, on0):
            box = {}

            def t_bursts():
                rb = ringp.tile([128, 512], F32, tag="ring", name="rb")
                box["rb"] = rb
                burst(rb[:, 0:256], pbts, s, 1)
                burst(rb[:, 256:512], pbts, s, 2)

            def t_norms():
                rb = box["rb"]
                ons = [on0]
                for ic, src in ((1, rb[:, 0:256]), (2, rb[:, 256:512])):
                    on = onp.tile([128, D], BF16, tag="on", name="on")
                    nc.vector.tensor_mul(on[:], src, recs[ic][:])
                    ons.append(on)
                box["ons"] = ons

            def t_transp(po_cur):
                ons = box["ons"]
                slots_a = po_cur[:, 0:256].bitcast(BF16)
                slots_b = po_cur[:, 280:408].bitcast(BF16)

                def slot(sl):
                    if sl < 4:
                        return slots_a[:, sl * 128:(sl + 1) * 128]
                    return slots_b[:, (sl - 4) * 128:(sl - 3) * 128]

                for ic in range(3):
                    for dk in range(KT):
                        nc.tensor.transpose(
                            slot(ic * 2 + dk),
                            ons[ic][:, dk * 128:(dk + 1) * 128], identb[:])
                ot = otp.tile([128, 3, KT, 128], BF16, tag="ot", name="ot")
                nc.vector.tensor_copy(
                    ot[:, 0:2].rearrange("p a b c -> p (a b c)"), slots_a[:])
                nc.vector.tensor_copy(
                    ot[:, 2].rearrange("p a b -> p (a b)"), slots_b[:])
                box["ot"] = ot
                box["fo"] = fop.tile([128, 3, D], F32, tag="fo", name="fo")

            def t_pf(ic, po_cur):
                ot, fo = box["ot"], box["fo"]
                pf = po_cur[:, 0:256]
                for dk in range(KT):
                    nc.tensor.matmul(pf, ot[:, ic, dk, :], woT[:, dk, :],
                                     start=(dk == 0), stop=(dk == KT - 1))
                if zero_bias:
                    nc.vector.tensor_copy(fo[:, ic, :], pf)
                else:
                    nc.vector.scalar_tensor_tensor(
                        fo[:, ic, :], pf, 1.0, bo_bc[:], ALU.mult, ALU.add)

            def t_dma():
                nc.sync.dma_start(
                    out=out_dram[s * R:(s + 1) * R, :]
                    .rearrange("(ic p) n -> p ic n", p=128),
                    in_=box["fo"][:])

            return {0: lambda po_cur: t_bursts(),
                    1: lambda po_cur: t_norms(),
                    2: lambda po_cur: t_transp(po_cur),
                    3: lambda po_cur: t_pf(0, po_cur),
                    4: lambda po_cur: t_pf(1, po_cur),
                    5: lambda po_cur: t_pf(2, po_cur),
                    6: lambda po_cur: t_dma()}, box

        # ---- attention rows ----
        def emit_L(s, h):
            b, g = 32 * (h % 4), h // 4
            lgt = lgp.tile([128, JT, 512], F32, tag="lg", name="lgt")
            for jt in range(JT):
                nc.tensor.matmul(
                    lgt[:, jt, 0:R],
                    qkT[b:b + 32, 2 + g,
                        s * R + jt * 128:s * R + (jt + 1) * 128],
                    qkT[b:b + 32, g, s * R:(s + 1) * R],
                    start=True, stop=True, tile_position=(b, 0))
            return lgt

        tail_sched, tail_box = {}, None
        lgt_next = emit_L(0, 0)
        for s in range(SS):
            po_t = pop.tile([128, 512], F32, tag="po", name="po_t")
            pbts = []
            for h in range(H):
                lgt = lgt_next
                pt = ptp.tile([128, JT, R], BF16, tag="pt", name="pt")
                nc.scalar.activation(pt[:], lgt[:, :, 0:R], AF.Exp)
                if (s, h) != (SS - 1, H - 1):
                    ns, nh = (s, h + 1) if h < H - 1 else (s + 1, 0)
                    lgt_next = emit_L(ns, nh)
                pbt = pbtp.tile([128, JT, R], BF16, tag="pbt", name="pbt")
                pbt_eng = nc.gpsimd if h in (1, 3, 5) else nc.vector
                pbt_eng.tensor_mul(pbt[:], pt[:], biasT[:, h])
                for ic in range(3):
                    col = 256 + ic * 8 + h
                    for jt in range(JT):
                        nc.tensor.matmul(
                            po_t[:, col:col + 1],
                            pt[:, jt, ic * 128:(ic + 1) * 128],
                            m01b[:, s, jt:jt + 1],
                            start=(jt == 0), stop=(jt == JT - 1))
                pbts.append(pbt)
                with tc.tile_wait_until(slot_floor(s, h)):
                    for fn in drip_sched.get((s, h), ()):
                        fn()
                    if h in tail_sched:
                        tail_sched.pop(h)(po_t)
                    if s == SS - 1 and h == 5:
                        rb3 = ringp.tile([128, 512], F32, tag="ring",
                                         name="rb3")
                        burst(po_t[:, 0:256], pbts, s, 0, 0, 6)
                        burst(rb3[:, 0:256], pbts, s, 1, 0, 6)
                        burst(rb3[:, 256:512], pbts, s, 2, 0, 6)
                    if s == SS - 1 and h == 6:
                        burst(po_t[:, 0:256], pbts, s, 0, 6, 7)
                        burst(rb3[:, 0:256], pbts, s, 1, 6, 7)
                        burst(rb3[:, 256:512], pbts, s, 2, 6, 7)
            # row end: reciprocals, o-chunk ic0, its normalize
            rf = recfp.tile([128, 24], F32, tag="rf", name="rf")
            nc.vector.reciprocal(rf[:], po_t[:, 256:280])
            recs = []
            for ic in range(3):
                re = recep.tile([128, D], F32, tag="re", name="re")
                nc.gpsimd.tensor_copy(
                    re[:].rearrange("p (h t) -> p h t", t=HD),
                    rf[:, ic * 8:(ic + 1) * 8]
                    .rearrange("p (h o) -> p h o", o=1)
                    .broadcast_to((128, 8, HD)))
                recs.append(re)
            if s == SS - 1:
                burst(po_t[:, 0:256], pbts, s, 0, 7, 8)
            else:
                burst(po_t[:, 0:256], pbts, s, 0)
            on0 = onp.tile([128, D], BF16, tag="on", name="on0")
            nc.vector.tensor_mul(on0[:], po_t[:, 0:256], recs[0][:])
            assert not tail_sched
            tail_sched, tail_box = make_tail(s, po_t, pbts, recs, on0)
            if s == SS - 1:
                # last row: finish bursts (head 7), then per-ic chains with
                # per-engine queues ordered by expected readiness
                tail_sched = {}
                rb = rb3
                burst(rb[:, 0:256], pbts, s, 1, 7, 8)
                burst(rb[:, 256:512], pbts, s, 2, 7, 8)
                srcs = (po_t[:, 0:256], rb[:, 0:256], rb[:, 256:512])
                on1 = onp.tile([128, D], BF16, tag="on", name="on1l")
                nc.vector.tensor_mul(on1[:], srcs[1], recs[1][:])
                on2 = onp.tile([128, D], BF16, tag="on", name="on2l")
                nc.vector.tensor_mul(on2[:], srcs[2], recs[2][:])
                ons = [on0, on1, on2]
                Tl = lgp.tile([128, JT, 512], F32, tag="lg", name="Tl")
                slots = Tl[:, 0, :].bitcast(BF16)
                ot = otp.tile([128, 3, KT, 128], BF16, tag="ot", name="otl")
                fo = fop.tile([128, 3, D], F32, tag="fo", name="fol")
                pf_tgts = (Tl[:, 1, 0:256], Tl[:, 2, 0:256], po_t[:, 0:256])

                def transp(ic):
                    for dk in range(KT):
                        sl = ic * 2 + dk
                        nc.tensor.transpose(
                            slots[:, sl * 128:(sl + 1) * 128],
                            ons[ic][:, dk * 128:(dk + 1) * 128], identb[:])

                def otcopy(ic, eng):
                    otv = ot[:, ic].rearrange("p a b -> p (a b)")
                    if eng is nc.scalar:
                        eng.copy(otv, slots[:, ic * 256:(ic + 1) * 256])
                    else:
                        eng.tensor_copy(otv, slots[:, ic * 256:(ic + 1) * 256])

                def pf_mm(ic):
                    for dk in range(KT):
                        nc.tensor.matmul(pf_tgts[ic], ot[:, ic, dk, :],
                                         woT[:, dk, :],
                                         start=(dk == 0), stop=(dk == KT - 1))

                def fo_cp(ic, eng):
                    if not zero_bias:
                        eng.scalar_tensor_tensor(
                            fo[:, ic, :], pf_tgts[ic], 1.0, bo_bc[:],
                            ALU.mult, ALU.add)
                    elif eng is nc.scalar:
                        eng.copy(fo[:, ic, :], pf_tgts[ic])
                    else:
                        eng.tensor_copy(fo[:, ic, :], pf_tgts[ic])

                def dma(ic, eng):
                    eng.dma_start(
                        out=out_dram[(s * JT + ic) * 128:
                                     (s * JT + ic + 1) * 128, :],
                        in_=fo[:, ic, :])

                # PE: all transposes first, then the out-proj matmuls
                transp(0)
                transp(1)
                transp(2)
                otcopy(0, nc.vector)    # DVE
                otcopy(1, nc.scalar)    # ACT
                otcopy(2, nc.scalar)    # ACT
                pf_mm(0)
                pf_mm(1)
                pf_mm(2)
                fo_cp(0, nc.scalar)
                fo_cp(1, nc.vector)
                fo_cp(2, nc.scalar)
                dma(0, nc.gpsimd)
                dma(1, nc.sync)
                dma(2, nc.scalar)
def make_in_maps(pair_act, attention_mask, bias, W_qkv, b_qkv, W_out, b_out):
    """Shard the full inputs across the 8 cores (data-parallel over S).

    All matmul operand transposes happen here on the host; the device sees
    flat [128, F] segments it can DMA without any on-device transposes.
    """
    bf = ml_dtypes.bfloat16
    x_all = np.asarray(pair_act, np.float32)[0]          # [S, R, D]
    mask01 = 1.0 - np.asarray(attention_mask, np.float32)[0]  # [S, R]
    b3 = np.asarray(bias, np.float32)[0, 0]              # [H, R, R]
    Wq = np.asarray(W_qkv, np.float32)                   # [3D, D]
    Wo = np.asarray(W_out, np.float32)                   # [D, D]

    wqT = Wq.T.reshape(KT, 128, 3 * D).transpose(1, 0, 2)    # [128,2,768]
    woT = Wo.T.reshape(KT, 128, D).transpose(1, 0, 2)        # [128,2,256]
    biasT = (b3.transpose(0, 2, 1)                           # [H, j, i]
             .reshape(H, JT, 128, R).transpose(2, 0, 1, 3))  # [128,H,3,384]
    bias_all = biasT.astype(bf).reshape(128, -1).ravel()  # [128, H*3*384]
    wq_bf = wqT.astype(bf).reshape(128, -1)
    wo_bf = woT.astype(bf).reshape(128, -1)
    identb = np.eye(128, dtype=np.float32).astype(bf)
    bqv = np.asarray(b_qkv, np.float32)
    bq6 = bqv[0:768].reshape(6, 128).T                       # [128, 6]

    in_maps = []
    for c in range(NCORES):
        xs = x_all[c * SS:(c + 1) * SS].reshape(M, D)        # [1536, 256]
        xT = xs.T.reshape(KT, 128, M).transpose(1, 0, 2)     # [128, 2, 1536]
        xrow = [xT[:, :, r_ * R:(r_ + 1) * R].astype(bf).reshape(128, -1)
                for r_ in range(SS)]
        m01 = mask01[c * SS:(c + 1) * SS]                    # [4, 384]
        m01p = m01.reshape(SS, JT, 128).transpose(2, 0, 1)   # [128, 4, 3]
        xb0 = np.concatenate([xrow[0],
                              m01p.astype(bf).reshape(128, -1)], axis=1)
        wx1 = np.concatenate(
            [xrow[1], xrow[2], xrow[3], wo_bf, identb], axis=1)
        allin_bf = np.concatenate(
            [wq_bf.ravel(), xb0.ravel(), wx1.ravel(), bias_all])
        f32seg = np.concatenate([m01p.reshape(128, -1), bq6], axis=1)
        allin_f32 = np.concatenate([
            f32seg.ravel().astype(np.float32),
            bqv[512:768], np.asarray(b_out, np.float32)])
        assert allin_bf.size == NBF and allin_f32.size == NF32, \
            (allin_bf.size, NBF, allin_f32.size, NF32)
        in_maps.append({
            "allin_bf": np.ascontiguousarray(allin_bf),
            "allin_f32": np.ascontiguousarray(allin_f32.astype(np.float32)),
        })
    return in_maps


_PROGRAM_CACHE = {}


def kernel(pair_act, attention_mask, bias, W_qkv, b_qkv, W_out, b_out,
           _want_results=False, **extra):
    in_maps = make_in_maps(pair_act, attention_mask, bias, W_qkv, b_qkv,
                           W_out, b_out)
    zero_bias = bool(np.all(np.asarray(b_qkv) == 0)
                     and np.all(np.asarray(b_out) == 0))
    key = ("nc", zero_bias)
    if key not in _PROGRAM_CACHE:
        _PROGRAM_CACHE[key] = build_program(zero_bias)
    nc = _PROGRAM_CACHE[key]
    res = run_bass_kernel_spmd(nc, in_maps, core_ids=list(range(NCORES)))
    out = np.concatenate(
        [r["out"].reshape(SS, R, D) for r in res.results], axis=0)
    out = out.reshape(B, S, R, D).astype(np.float32)
    if _want_results:
        return out, res
    return out
